# revision 9
# baseline (speedup 1.0000x reference)
"""Multi-head self-attention (B=2, S=2048, d_model=1024, H=16, RoPE, causal)
on 8 Trainium2 NeuronCores, tensor-parallel over heads (2 heads/core).

Layout strategy (per core c, heads 2c and 2c+1):
  - host pre-transposes x -> xT [1024, 4096] (tokens = B*S flattened) and
    builds per-core transposed weight slices; Wq/Wk rows are de-interleaved
    per head ([evens, odds]) so RoPE becomes block-structured.
  - q/k/v are produced transposed ([dims, tok]); RoPE applied with
    precomputed cos/sin tables (partner swap via SBUF->SBUF DMA, math on
    DVE at full batch width); V is re-transposed to natural layout with an
    appended ones-block so the P@V matmul also produces the softmax
    denominator (replicated over 64 partitions).
  - scores are computed transposed (S^T[k, q]) per 128-wide k-tile with the
    two heads packed into the PE array via tile_position row tiling
    (contraction = 64 head dims each); exp runs on the scalar engine
    straight out of PSUM with the 1/sqrt(64) scale folded in; causality is
    handled by only visiting valid (k-tile, q-range) spans plus an additive
    -1e30 bias on the diagonal 128x128 block.
  - attention outputs (divided by the denominator) are AllGathered across
    the 8 cores in 1024-token chunks (0.5MB/rank) so gathers overlap later
    compute; each core then computes its 128 output dims of the final
    projection. Host concatenates + transposes.

All matmuls run in float32r (TF32-like, 1 cycle/row at free-dim >= 256,
measured rel-err ~1.5e-4 for K=1024 dots). Elementwise work stays float32;
tensors feeding the PE are declared/produced as float32r. Matmul outputs
never straddle PSUM bank (512-col) boundaries.
"""

import ml_dtypes
import numpy as np

import concourse.bass as bass
import concourse.mybir as mybir
import concourse.tile as tile
from concourse import bacc
from concourse.bass_utils import run_bass_kernel_spmd

P = 128
B, S, D = 2, 2048, 1024
T = B * S          # 4096 flattened tokens
H = 16
DH = 64            # head dim
NC = 8             # cores
HPC = H // NC      # heads per core = 2
DPC = HPC * DH     # dims per core = 128
KT = D // P        # 8 contraction tiles for d_model
TB = 512           # token block for projections
G = 512            # attention q-group width
AGW = 1024         # AllGather chunk width (tokens)
ROPE_THETA = 10000.0

F32 = mybir.dt.float32
R = mybir.dt.float32r
BF = mybir.dt.bfloat16

_CACHE = {}


def _build():
    nc = bacc.Bacc(None, target_bir_lowering=False)

    xT = nc.dram_tensor("xT", [D, T], BF, kind="ExternalInput")
    wq = nc.dram_tensor("wq", [D, DPC], BF, kind="ExternalInput")
    wk = nc.dram_tensor("wk", [D, DPC], BF, kind="ExternalInput")
    wv = nc.dram_tensor("wv", [D, DPC], BF, kind="ExternalInput")
    wo = nc.dram_tensor("wo", [D, DPC], BF, kind="ExternalInput")
    cosb = nc.dram_tensor("cosb", [P, S], F32, kind="ExternalInput")
    sinb = nc.dram_tensor("sinb", [P, S], F32, kind="ExternalInput")
    maskb = nc.dram_tensor("maskb", [P, P], F32, kind="ExternalInput")
    iden = nc.dram_tensor("iden", [P, P], R, kind="ExternalInput")
    ones = nc.dram_tensor("ones", [P, DH], R, kind="ExternalInput")
    outT = nc.dram_tensor("outT", [DPC, T], R, kind="ExternalOutput")

    with tile.TileContext(nc) as tc:
        with (
            tc.tile_pool(name="cst", bufs=1) as cst,
            tc.tile_pool(name="wpool", bufs=1) as wpool,
            tc.tile_pool(name="xin", bufs=2) as xin,
            tc.tile_pool(name="qk", bufs=1) as qkpool,
            tc.tile_pool(name="vxp", bufs=1) as vxp,
            tc.tile_pool(name="tmp", bufs=1) as tmp,
            tc.tile_pool(name="pt", bufs=3) as ptpool,
            tc.tile_pool(name="att", bufs=2) as attp,
            tc.tile_pool(name="prj", bufs=3) as prj,
            tc.tile_pool(name="ps", bufs=4, space="PSUM") as ps,
            tc.tile_pool(name="dram", bufs=1, space="DRAM") as dram,
        ):
            # ---- constants ----
            cos_t = cst.tile([P, S], F32)
            sin_t = cst.tile([P, S], F32)
            mk = cst.tile([P, P], F32)
            idn = cst.tile([P, P], R)
            nc.sync.dma_start(cos_t, cosb[:, :])
            nc.sync.dma_start(sin_t, sinb[:, :])
            nc.sync.dma_start(mk, maskb[:, :])
            nc.sync.dma_start(idn, iden[:, :])

            ws = {}
            for name, w in (("q", wq), ("k", wk), ("v", wv), ("o", wo)):
                wt = wpool.tile([P, KT, DPC], BF, name=f"w{name}")
                wr = w.rearrange("(ko p) m -> ko p m", p=P)
                for k in range(KT):
                    nc.sync.dma_start(wt[:, k], wr[k])
                ws[name] = wt

            ag_in = [
                [
                    dram.tile([DPC, AGW], BF, name=f"agin{b}_{a}")
                    for a in range(S // AGW)
                ]
                for b in range(B)
            ]
            ag_out = [
                [
                    dram.tile(
                        [NC * DPC, AGW], BF, addr_space="Shared", name=f"agout{b}_{a}"
                    )
                    for a in range(S // AGW)
                ]
                for b in range(B)
            ]

            xTr = xT.rearrange("(ko p) t -> ko p t", p=P)

            def qkv_phase(b):
                """Project x -> qT/kT (roped) and build V_ext tiles."""
                T0 = S * b
                qT = qkpool.tile([P, S], R, name="qT", tag="qT")
                kT = qkpool.tile([P, S], R, name="kT", tag="kT")
                vraw = qkpool.tile([P, S], R, name="vraw", tag="vraw")
                qraw = tmp.tile([P, S], F32, name="qraw", tag="qraw")
                kraw = tmp.tile([P, S], F32, name="kraw", tag="kraw")
                vx = [
                    vxp.tile([P, S // P, P], R, name=f"vx{h}", tag=f"vx{h}")
                    for h in range(HPC)
                ]
                ones_b = bass.AP(
                    tensor=ones, offset=0, ap=[[DH, P], [0, S // P], [1, DH]]
                )
                for h in range(HPC):
                    nc.sync.dma_start(vx[h][:, :, DH:], ones_b)

                for j in range(S // TB):
                    xblk = xin.tile([P, KT, TB], BF, name="xblk", tag="xblk")
                    for k in range(KT):
                        nc.sync.dma_start(
                            xblk[:, k], xTr[k, :, T0 + j * TB : T0 + (j + 1) * TB]
                        )
                    js = slice(j * TB, (j + 1) * TB)
                    for name, dst in (("q", qraw), ("k", kraw), ("v", vraw)):
                        pp = ps.tile([P, TB], F32, name=f"pp{name}", tag="ps")
                        for k in range(KT):
                            nc.tensor.matmul(
                                pp,
                                ws[name][:, k],
                                xblk[:, k],
                                start=(k == 0),
                                stop=(k == KT - 1),
                            )
                        if name == "v":
                            nc.vector.tensor_copy(dst[:, js], pp)
                        else:
                            nc.scalar.copy(dst[:, js], pp)

                # RoPE at full batch width: out = raw*cos + swap(raw)*sin'
                for raw, dst in ((qraw, qT), (kraw, kT)):
                    g = tmp.tile([P, S], F32, name="g", tag="g")
                    for s0, s1 in ((0, 32), (32, 0), (64, 96), (96, 64)):
                        nc.sync.dma_start(g[s0 : s0 + 32], raw[s1 : s1 + 32])
                    t1 = tmp.tile([P, S], F32, name="t1", tag="t1")
                    nc.vector.tensor_tensor(t1, raw, cos_t, mybir.AluOpType.mult)
                    t2 = tmp.tile([P, S], F32, name="t2", tag="t2")
                    nc.vector.tensor_tensor(t2, g, sin_t, mybir.AluOpType.mult)
                    nc.vector.tensor_tensor(dst, t1, t2, mybir.AluOpType.add)

                # V transpose: [dims, tok] -> [tok, dims] per 128-token tile
                for t in range(S // P):
                    vtp = ps.tile([P, P], F32, name="vtp", tag="ps")
                    nc.tensor.transpose(
                        vtp.bitcast(R), vraw[:, t * P : (t + 1) * P], idn
                    )
                    for h in range(HPC):
                        nc.scalar.copy(
                            vx[h][:, t, 0:DH], vtp[:, DH * h : DH * (h + 1)]
                        )
                return qT, kT, vx

            def attention_qgroup(b, g, qT, kT, vx):
                """Causal attention for q-cols [g*G, (g+1)*G) of batch b."""
                oa = ps.tile([P, HPC, G], F32, name="oa", tag="ps")
                n_t = (g + 1) * G // P  # valid k-tiles
                for t in range(n_t):
                    c0 = max(0, t * P - g * G)
                    sc = ps.tile([P, HPC, G], F32, name="sc", tag="ps")
                    for h in range(HPC):
                        hs = slice(DH * h, DH * (h + 1))
                        nc.tensor.matmul(
                            sc[:, h, c0:],
                            kT[hs, t * P : (t + 1) * P],
                            qT[hs, g * G + c0 : (g + 1) * G],
                            start=True,
                            stop=True,
                            tile_position=(DH * h, 0),
                        )
                        if t * P >= g * G:  # diagonal block: causal bias
                            nc.vector.tensor_tensor(
                                sc[:, h, c0 : c0 + P],
                                sc[:, h, c0 : c0 + P],
                                mk,
                                mybir.AluOpType.add,
                            )
                    pT = ptpool.tile([P, HPC, G], R, name="pT", tag="pT")
                    nc.scalar.activation(
                        pT[:, :, c0:],
                        sc[:, :, c0:],
                        mybir.ActivationFunctionType.Exp,
                        scale=1.0 / np.sqrt(DH),
                    )
                    for h in range(HPC):
                        nc.tensor.matmul(
                            oa[:, h, c0:],
                            vx[h][:, t],
                            pT[:, h, c0:],
                            start=(t == 0),
                            stop=(t == n_t - 1),
                            skip_group_check=True,
                        )
                # divide by denominator (rows DH:2*DH = replicated sum)
                for h in range(HPC):
                    rec = attp.tile([DH, G], F32, name="rec", tag="rec")
                    nc.vector.reciprocal(rec, oa[DH:P, h])
                    at = attp.tile([DH, G], BF, name="at", tag="at")
                    nc.vector.tensor_tensor(
                        at, oa[0:DH, h], rec, mybir.AluOpType.mult
                    )
                    a, gg = divmod(g, AGW // G)
                    nc.sync.dma_start(
                        ag_in[b][a][
                            DH * h : DH * (h + 1), gg * G : (gg + 1) * G
                        ],
                        at,
                    )

            def proj_chunk(b, a):
                """outT tokens [a*AGW, (a+1)*AGW) of batch b from ag_out."""
                T0 = S * b
                agr = ag_out[b][a].rearrange("(ko p) t -> ko p t", p=P)
                for j in range(a * AGW // TB, (a + 1) * AGW // TB):
                    po = ps.tile([P, TB], F32, name="po", tag="ps")
                    jj = j - a * AGW // TB
                    for k in range(KT):
                        rhs = prj.tile([P, TB], BF, name="rhs", tag="rhs", bufs=6)
                        nc.sync.dma_start(rhs, agr[k, :, jj * TB : (jj + 1) * TB])
                        nc.tensor.matmul(
                            po,
                            ws["o"][:, k],
                            rhs,
                            start=(k == 0),
                            stop=(k == KT - 1),
                        )
                    ob = prj.tile([P, TB], R, name="ob", tag="ob")
                    nc.vector.tensor_copy(ob, po)
                    nc.sync.dma_start(
                        outT[:, T0 + j * TB : T0 + (j + 1) * TB], ob
                    )

            n_ag = S // AGW  # AG chunks per batch
            for b in range(B):
                qT, kT, vx = qkv_phase(b)
                for a in range(n_ag):
                    for g in range(a * AGW // G, (a + 1) * AGW // G):
                        attention_qgroup(b, g, qT, kT, vx)
                    nc.gpsimd.collective_compute(
                        "AllGather",
                        mybir.AluOpType.bypass,
                        replica_groups=[list(range(NC))],
                        ins=[ag_in[b][a][:, :]],
                        outs=[ag_out[b][a][:, :]],
                    )
                    proj_chunk(b, a)

    nc.compile()
    return nc


def _host_inputs(x, token_positions, Wq, Wk, Wv, Wo):
    xT = np.ascontiguousarray(x.reshape(T, D).T).astype(ml_dtypes.bfloat16)  # [D, T]

    # de-interleave perm within each 64-dim head: [evens, odds]
    perm = np.concatenate(
        [64 * h + np.r_[np.arange(0, 64, 2), np.arange(1, 64, 2)] for h in range(HPC)]
    )

    pos = token_positions.astype(np.float64)  # [S]
    inv_freq = ROPE_THETA ** (-np.arange(0, DH, 2, dtype=np.float64) / DH)  # [32]
    ang = pos[:, None] * inv_freq[None, :]  # [S, 32]
    cos = np.cos(ang).T.astype(np.float32)  # [32, S]
    sin = np.sin(ang).T.astype(np.float32)
    cosb = np.concatenate([cos, cos, cos, cos], axis=0)  # [128, S]
    sinb = np.concatenate([-sin, sin, -sin, sin], axis=0)

    maskb = np.where(np.triu(np.ones((P, P), dtype=bool)), 0.0, -1e30).astype(
        np.float32
    )
    iden = np.eye(P, dtype=np.float32)
    ones = np.ones((P, DH), dtype=np.float32)

    in_maps = []
    for c in range(NC):
        rs = slice(DPC * c, DPC * (c + 1))
        in_maps.append(
            {
                "xT": xT,
                "wq": np.ascontiguousarray(Wq[rs][perm].T).astype(ml_dtypes.bfloat16),
                "wk": np.ascontiguousarray(Wk[rs][perm].T).astype(ml_dtypes.bfloat16),
                "wv": np.ascontiguousarray(Wv[rs].T).astype(ml_dtypes.bfloat16),
                "wo": np.ascontiguousarray(Wo[rs].T).astype(ml_dtypes.bfloat16),
                "cosb": cosb,
                "sinb": sinb,
                "maskb": maskb,
                "iden": iden,
                "ones": ones,
            }
        )
    return in_maps


def kernel(x, token_positions, Wq, Wk, Wv, Wo, _trace=False, _result=[None]):
    x = np.asarray(x, dtype=np.float32)
    token_positions = np.asarray(token_positions)
    Wq, Wk, Wv, Wo = (np.asarray(w, dtype=np.float32) for w in (Wq, Wk, Wv, Wo))

    if "nc" not in _CACHE:
        _CACHE["nc"] = _build()
    nc = _CACHE["nc"]

    in_maps = _host_inputs(x, token_positions, Wq, Wk, Wv, Wo)
    res = run_bass_kernel_spmd(nc, in_maps, core_ids=list(range(NC)), trace=_trace)
    _result[0] = res
    full_T = np.concatenate([res.results[c]["outT"] for c in range(NC)], axis=0)
    return np.ascontiguousarray(full_T.T).reshape(B, S, D)


# revision 10
# speedup vs baseline: 1.1796x; 1.1796x over previous
"""Multi-head self-attention (B=2, S=2048, d_model=1024, H=16, RoPE, causal)
on 8 Trainium2 NeuronCores, tensor-parallel over heads (2 heads/core).

Layout strategy (per core c, heads 2c and 2c+1):
  - host pre-transposes x -> xT [1024, 4096] (tokens = B*S flattened) and
    builds per-core transposed weight slices; Wq/Wk rows are de-interleaved
    per head ([evens, odds]) so RoPE becomes block-structured.
  - q/k/v are produced transposed ([dims, tok]); RoPE applied with
    precomputed cos/sin tables (partner swap via SBUF->SBUF DMA, math on
    DVE at full batch width); V is re-transposed to natural layout with an
    appended ones-block so the P@V matmul also produces the softmax
    denominator (replicated over 64 partitions).
  - scores are computed transposed (S^T[k, q]) per 128-wide k-tile with the
    two heads packed into the PE array via tile_position row tiling
    (contraction = 64 head dims each); exp runs on the scalar engine
    straight out of PSUM with the 1/sqrt(64) scale folded in; causality is
    handled by only visiting valid (k-tile, q-range) spans plus an additive
    -1e30 bias on the diagonal 128x128 block.
  - attention outputs (divided by the denominator) are AllGathered across
    the 8 cores in 1024-token chunks (0.5MB/rank) so gathers overlap later
    compute; each core then computes its 128 output dims of the final
    projection. Host concatenates + transposes.

All matmuls run in float32r (TF32-like, 1 cycle/row at free-dim >= 256,
measured rel-err ~1.5e-4 for K=1024 dots). Elementwise work stays float32;
tensors feeding the PE are declared/produced as float32r. Matmul outputs
never straddle PSUM bank (512-col) boundaries.
"""

import ml_dtypes
import numpy as np

import concourse.bass as bass
import concourse.mybir as mybir
import concourse.tile as tile
from concourse import bacc
from concourse.bass_utils import run_bass_kernel_spmd

P = 128
B, S, D = 2, 2048, 1024
T = B * S          # 4096 flattened tokens
H = 16
DH = 64            # head dim
NC = 8             # cores
HPC = H // NC      # heads per core = 2
DPC = HPC * DH     # dims per core = 128
KT = D // P        # 8 contraction tiles for d_model
TB = 512           # token block for projections
G = 512            # attention q-group width
AGW = 1024         # AllGather chunk width (tokens)
ROPE_THETA = 10000.0

F32 = mybir.dt.float32
R = mybir.dt.float32r
BF = mybir.dt.bfloat16

_CACHE = {}


def _build():
    nc = bacc.Bacc(None, target_bir_lowering=False)

    xT = nc.dram_tensor("xT", [D, T], BF, kind="ExternalInput")
    wq = nc.dram_tensor("wq", [D, DPC], BF, kind="ExternalInput")
    wk = nc.dram_tensor("wk", [D, DPC], BF, kind="ExternalInput")
    wv = nc.dram_tensor("wv", [D, DPC], BF, kind="ExternalInput")
    wo = nc.dram_tensor("wo", [D, DPC], BF, kind="ExternalInput")
    cosb = nc.dram_tensor("cosb", [P, S], F32, kind="ExternalInput")
    sinb = nc.dram_tensor("sinb", [P, S], F32, kind="ExternalInput")
    maskb = nc.dram_tensor("maskb", [P, P], F32, kind="ExternalInput")
    iden = nc.dram_tensor("iden", [P, P], R, kind="ExternalInput")
    ones = nc.dram_tensor("ones", [P, DH], R, kind="ExternalInput")
    outT = nc.dram_tensor("outT", [DPC, T], R, kind="ExternalOutput")

    with tile.TileContext(nc) as tc:
        with (
            tc.tile_pool(name="cst", bufs=1) as cst,
            tc.tile_pool(name="wpool", bufs=1) as wpool,
            tc.tile_pool(name="xin", bufs=2) as xin,
            tc.tile_pool(name="qk", bufs=1) as qkpool,
            tc.tile_pool(name="vxp", bufs=1) as vxp,
            tc.tile_pool(name="tmp", bufs=1) as tmp,
            tc.tile_pool(name="pt", bufs=3) as ptpool,
            tc.tile_pool(name="att", bufs=2) as attp,
            tc.tile_pool(name="prj", bufs=3) as prj,
            tc.tile_pool(name="ps", bufs=4, space="PSUM") as ps,
            tc.tile_pool(name="dram", bufs=1, space="DRAM") as dram,
        ):
            # ---- constants ----
            cos_t = cst.tile([P, S], F32)
            sin_t = cst.tile([P, S], F32)
            mk = cst.tile([P, P], F32)
            idn = cst.tile([P, P], R)
            nc.sync.dma_start(cos_t, cosb[:, :])
            nc.sync.dma_start(sin_t, sinb[:, :])
            nc.sync.dma_start(mk, maskb[:, :])
            nc.sync.dma_start(idn, iden[:, :])

            ws = {}
            for name, w in (("q", wq), ("k", wk), ("v", wv), ("o", wo)):
                wt = wpool.tile([P, KT, DPC], BF, name=f"w{name}")
                wr = w.rearrange("(ko p) m -> ko p m", p=P)
                for k in range(KT):
                    nc.sync.dma_start(wt[:, k], wr[k])
                ws[name] = wt

            ag_in = [
                [
                    dram.tile([DPC, AGW], BF, name=f"agin{b}_{a}")
                    for a in range(S // AGW)
                ]
                for b in range(B)
            ]
            ag_out = [
                [
                    dram.tile(
                        [NC * DPC, AGW], BF, addr_space="Shared", name=f"agout{b}_{a}"
                    )
                    for a in range(S // AGW)
                ]
                for b in range(B)
            ]

            xTr = xT.rearrange("(ko p) t -> ko p t", p=P)

            def qkv_phase(b):
                """Project x -> qT/kT (roped) and build V_ext tiles."""
                T0 = S * b
                qT = qkpool.tile([P, S], R, name="qT", tag="qT")
                kT = qkpool.tile([P, S], R, name="kT", tag="kT")
                vraw = qkpool.tile([P, S], R, name="vraw", tag="vraw")
                qraw = tmp.tile([P, S], F32, name="qraw", tag="qraw")
                kraw = tmp.tile([P, S], F32, name="kraw", tag="kraw")
                vx = [
                    vxp.tile([P, S // P, P], R, name=f"vx{h}", tag=f"vx{h}")
                    for h in range(HPC)
                ]
                ones_b = bass.AP(
                    tensor=ones, offset=0, ap=[[DH, P], [0, S // P], [1, DH]]
                )
                for h in range(HPC):
                    nc.sync.dma_start(vx[h][:, :, DH:], ones_b)

                for j in range(S // TB):
                    xblk = xin.tile([P, KT, TB], BF, name="xblk", tag="xblk", bufs=3)
                    for k in range(KT):
                        nc.sync.dma_start(
                            xblk[:, k], xTr[k, :, T0 + j * TB : T0 + (j + 1) * TB]
                        )
                    js = slice(j * TB, (j + 1) * TB)
                    for name, dst in (("q", qraw), ("k", kraw), ("v", vraw)):
                        pp = ps.tile([P, TB], F32, name=f"pp{name}", tag="ps")
                        for k in range(KT):
                            nc.tensor.matmul(
                                pp,
                                ws[name][:, k],
                                xblk[:, k],
                                start=(k == 0),
                                stop=(k == KT - 1),
                            )
                        if name == "v":
                            nc.vector.tensor_copy(dst[:, js], pp)
                        else:
                            nc.scalar.copy(dst[:, js], pp)

                # RoPE at full batch width: out = raw*cos + swap(raw)*sin'
                for raw, dst in ((qraw, qT), (kraw, kT)):
                    g = tmp.tile([P, S], F32, name="g", tag="g")
                    for s0, s1 in ((0, 32), (32, 0), (64, 96), (96, 64)):
                        nc.sync.dma_start(g[s0 : s0 + 32], raw[s1 : s1 + 32])
                    t1 = tmp.tile([P, S], F32, name="t1", tag="t1")
                    nc.vector.tensor_tensor(t1, raw, cos_t, mybir.AluOpType.mult)
                    t2 = tmp.tile([P, S], F32, name="t2", tag="t2")
                    nc.vector.tensor_tensor(t2, g, sin_t, mybir.AluOpType.mult)
                    nc.vector.tensor_tensor(dst, t1, t2, mybir.AluOpType.add)

                # V transpose: [dims, tok] -> [tok, dims] per 128-token tile
                for t in range(S // P):
                    vtp = ps.tile([P, P], F32, name="vtp", tag="ps")
                    nc.tensor.transpose(
                        vtp.bitcast(R), vraw[:, t * P : (t + 1) * P], idn
                    )
                    for h in range(HPC):
                        nc.scalar.copy(
                            vx[h][:, t, 0:DH], vtp[:, DH * h : DH * (h + 1)]
                        )
                return qT, kT, vx

            def attention_qgroup(b, g, qT, kT, vx):
                """Causal attention for q-cols [g*G, (g+1)*G) of batch b."""
                oa = ps.tile([P, HPC, G], F32, name="oa", tag="ps")
                n_t = (g + 1) * G // P  # valid k-tiles
                for t in range(n_t):
                    c0 = max(0, t * P - g * G)
                    sc = ps.tile([P, HPC, G], F32, name="sc", tag="ps")
                    for h in range(HPC):
                        hs = slice(DH * h, DH * (h + 1))
                        nc.tensor.matmul(
                            sc[:, h, c0:],
                            kT[hs, t * P : (t + 1) * P],
                            qT[hs, g * G + c0 : (g + 1) * G],
                            start=True,
                            stop=True,
                            tile_position=(DH * h, 0),
                        )
                        if t * P >= g * G:  # diagonal block: causal bias
                            nc.vector.tensor_tensor(
                                sc[:, h, c0 : c0 + P],
                                sc[:, h, c0 : c0 + P],
                                mk,
                                mybir.AluOpType.add,
                            )
                    pT = ptpool.tile([P, HPC, G], R, name="pT", tag="pT")
                    nc.scalar.activation(
                        pT[:, :, c0:],
                        sc[:, :, c0:],
                        mybir.ActivationFunctionType.Exp,
                        scale=1.0 / np.sqrt(DH),
                    )
                    for h in range(HPC):
                        nc.tensor.matmul(
                            oa[:, h, c0:],
                            vx[h][:, t],
                            pT[:, h, c0:],
                            start=(t == 0),
                            stop=(t == n_t - 1),
                            skip_group_check=True,
                        )
                # divide by denominator (rows DH:2*DH = replicated sum)
                for h in range(HPC):
                    rec = attp.tile([DH, G], F32, name="rec", tag="rec")
                    nc.vector.reciprocal(rec, oa[DH:P, h])
                    at = attp.tile([DH, G], BF, name="at", tag="at")
                    nc.vector.tensor_tensor(
                        at, oa[0:DH, h], rec, mybir.AluOpType.mult
                    )
                    a, gg = divmod(g, AGW // G)
                    nc.sync.dma_start(
                        ag_in[b][a][
                            DH * h : DH * (h + 1), gg * G : (gg + 1) * G
                        ],
                        at,
                    )

            def proj_chunk(b, a):
                """outT tokens [a*AGW, (a+1)*AGW) of batch b from ag_out."""
                T0 = S * b
                agr = ag_out[b][a].rearrange("(ko p) t -> ko p t", p=P)
                for j in range(a * AGW // TB, (a + 1) * AGW // TB):
                    po = ps.tile([P, TB], F32, name="po", tag="ps")
                    jj = j - a * AGW // TB
                    for k in range(KT):
                        rhs = prj.tile([P, TB], BF, name="rhs", tag="rhs", bufs=6)
                        nc.sync.dma_start(rhs, agr[k, :, jj * TB : (jj + 1) * TB])
                        nc.tensor.matmul(
                            po,
                            ws["o"][:, k],
                            rhs,
                            start=(k == 0),
                            stop=(k == KT - 1),
                        )
                    ob = prj.tile([P, TB], R, name="ob", tag="ob")
                    nc.vector.tensor_copy(ob, po)
                    nc.sync.dma_start(
                        outT[:, T0 + j * TB : T0 + (j + 1) * TB], ob
                    )

            n_ag = S // AGW  # AG chunks per batch
            for b in range(B):
                qT, kT, vx = qkv_phase(b)
                for a in range(n_ag):
                    for g in range(a * AGW // G, (a + 1) * AGW // G):
                        attention_qgroup(b, g, qT, kT, vx)
                    nc.gpsimd.collective_compute(
                        "AllGather",
                        mybir.AluOpType.bypass,
                        replica_groups=[list(range(NC))],
                        ins=[ag_in[b][a][:, :]],
                        outs=[ag_out[b][a][:, :]],
                    )
                for a in range(n_ag):
                    proj_chunk(b, a)

    nc.compile()
    return nc


def _host_inputs(x, token_positions, Wq, Wk, Wv, Wo):
    xT = np.ascontiguousarray(x.reshape(T, D).T).astype(ml_dtypes.bfloat16)  # [D, T]

    # de-interleave perm within each 64-dim head: [evens, odds]
    perm = np.concatenate(
        [64 * h + np.r_[np.arange(0, 64, 2), np.arange(1, 64, 2)] for h in range(HPC)]
    )

    pos = token_positions.astype(np.float64)  # [S]
    inv_freq = ROPE_THETA ** (-np.arange(0, DH, 2, dtype=np.float64) / DH)  # [32]
    ang = pos[:, None] * inv_freq[None, :]  # [S, 32]
    cos = np.cos(ang).T.astype(np.float32)  # [32, S]
    sin = np.sin(ang).T.astype(np.float32)
    cosb = np.concatenate([cos, cos, cos, cos], axis=0)  # [128, S]
    sinb = np.concatenate([-sin, sin, -sin, sin], axis=0)

    maskb = np.where(np.triu(np.ones((P, P), dtype=bool)), 0.0, -1e30).astype(
        np.float32
    )
    iden = np.eye(P, dtype=np.float32)
    ones = np.ones((P, DH), dtype=np.float32)

    in_maps = []
    for c in range(NC):
        rs = slice(DPC * c, DPC * (c + 1))
        in_maps.append(
            {
                "xT": xT,
                "wq": np.ascontiguousarray(Wq[rs][perm].T).astype(ml_dtypes.bfloat16),
                "wk": np.ascontiguousarray(Wk[rs][perm].T).astype(ml_dtypes.bfloat16),
                "wv": np.ascontiguousarray(Wv[rs].T).astype(ml_dtypes.bfloat16),
                "wo": np.ascontiguousarray(Wo[rs].T).astype(ml_dtypes.bfloat16),
                "cosb": cosb,
                "sinb": sinb,
                "maskb": maskb,
                "iden": iden,
                "ones": ones,
            }
        )
    return in_maps


def kernel(x, token_positions, Wq, Wk, Wv, Wo, _trace=False, _result=[None]):
    x = np.asarray(x, dtype=np.float32)
    token_positions = np.asarray(token_positions)
    Wq, Wk, Wv, Wo = (np.asarray(w, dtype=np.float32) for w in (Wq, Wk, Wv, Wo))

    if "nc" not in _CACHE:
        _CACHE["nc"] = _build()
    nc = _CACHE["nc"]

    in_maps = _host_inputs(x, token_positions, Wq, Wk, Wv, Wo)
    res = run_bass_kernel_spmd(nc, in_maps, core_ids=list(range(NC)), trace=_trace)
    _result[0] = res
    full_T = np.concatenate([res.results[c]["outT"] for c in range(NC)], axis=0)
    return np.ascontiguousarray(full_T.T).reshape(B, S, D)
